# revision 32
# baseline (speedup 1.0000x reference)
"""Expert-parallel fused MoE kernel for Trainium2 (8 NeuronCores).

Problem: B=2, T=1024, H=1024, F=1024, E=8 experts, top-2 routing.
N = B*T = 2048 tokens.

Strategy (expert parallel, one expert per core):
  - Host pre-transposes/pre-tiles inputs. Every core receives the full
    hidden state (transposed, [H, N]), ONE expert's weights, the (shared)
    router weight, its own 256-token slice of the hidden state for the
    sharded router, and a one-hot selector for its expert.
  - Router is data-parallel: each core computes fp32 logits for its 256
    tokens (fp32 keeps top-2 selection exact: the tightest top2/top3 logit
    gap is ~0.02 while float32r rounding would inject ~0.02 noise), then an
    AllGather shares all [E, N] logits with every core. Each core derives
    its expert's combine weight
        w = exp(l_e - m1) * [l_e >= m2] / (1 + exp(m2 - m1))
    (softmax + top-2 + renormalize collapses to this closed form; l_e is
    extracted with the one-hot selector).
  - The dense SwiGLU FFN runs over all 2048 tokens with float32r matmuls
    (full-rate fp32 on the PE; raw fp32 bits are passed, the PE rounds
    internally), output scaled by w -> partial output [N, H].
  - Chunked ReduceScatter(add) over the 8 cores reduces partial outputs per
    512-token chunk (overlapping compute); host reassembles the shards.

Matmul layouts (PE computes out = lhsT.T @ rhs, contraction on partitions):
  gate/up : lhsT = GU_T[f_tile][h_tile] (128x128), rhs = X_T[h_tile]
            (128x512) -> psum[f 128, n 512], accumulate over 8 h_tiles.
  down    : lhsT = act_T[f_tile][:, n_sub] (128x128), rhs = DP_T[f_tile]
            (128x512) -> psum[n 128, h' 512], accumulate over 8 f_tiles.
            (token-major output so the per-token scale w is a per-partition
            scalar fused into the PSUM->SBUF copy on the scalar engine)
  router  : lhsT = Wr_T[h_tile] (128x8) fp32, rhs = Xme_T[h_tile] (128x256)
            -> psum[8, 256]; AllGather -> [E, N]; PE-transposed token-major.
"""

import numpy as np

import concourse.bass as bass
import concourse.mybir as mybir
import concourse.tile as tile
from concourse import bacc
from concourse.bass_utils import run_bass_kernel_spmd
from concourse.masks import make_identity

P = 128
H = 1024
F = 1024
E = 8
N = 2048
HT = H // P          # 8 h tiles
FT = F // P          # 8 f tiles
NCHUNK = 4           # token chunks
CN = N // NCHUNK     # 512 tokens per chunk
NSUB = CN // P       # 4 token subtiles per chunk
NT = N // P          # 16 token tiles total
ME = N // 8          # 256 tokens per core for the sharded router
F32 = mybir.dt.float32
F32R = mybir.dt.float32r


def build_nc():
    nc = bacc.Bacc(None, target_bir_lowering=False)

    xt = nc.dram_tensor("xt", [H, N], F32, kind="ExternalInput")
    xme = nc.dram_tensor("xme", [H, ME], F32, kind="ExternalInput")
    wrt = nc.dram_tensor("wrt", [H, E], F32, kind="ExternalInput")
    sel = nc.dram_tensor("sel", [P, E], F32, kind="ExternalInput")
    # pre-tiled on host: [2*FT, P(h), HT, P(f)]; raw fp32 bits tagged f32r
    gut = nc.dram_tensor("gut", [2 * FT, P, HT, P], F32R, kind="ExternalInput")
    dpt = nc.dram_tensor("dpt", [F, H], F32R, kind="ExternalInput")
    out = nc.dram_tensor("out", [N // 8, H], F32, kind="ExternalOutput")

    xt_r = xt.rearrange("(hh p) n -> hh p n", p=P)
    xme_r = xme.rearrange("(hh p) n -> hh p n", p=P)
    dpt_r = dpt.rearrange("(ff p) h -> ff p h", p=P)
    wrt_r = wrt.rearrange("(hh p) e -> hh p e", p=P)

    with tile.TileContext(nc) as tc:
        with (
            tc.tile_pool(name="singles", bufs=1) as singles,
            tc.tile_pool(name="xtr", bufs=3) as xtr_pool,
            tc.tile_pool(name="actp", bufs=2) as act_pool,
            tc.tile_pool(name="sg", bufs=3) as sg_pool,
            tc.tile_pool(name="yp", bufs=4) as y_pool,
            tc.tile_pool(name="rsm", bufs=1) as rp,
            tc.tile_pool(name="lfp", bufs=4) as lf_pool,
            tc.tile_pool(name="gps", bufs=2, space="PSUM") as g_pool,
            tc.tile_pool(name="ups", bufs=2, space="PSUM") as u_pool,
            tc.tile_pool(name="dps", bufs=2, space="PSUM") as d_pool,
            tc.tile_pool(name="rps", bufs=1, space="PSUM") as r_pool,
            tc.tile_pool(name="tps", bufs=1, space="PSUM") as t_pool,
            tc.tile_pool(name="dram", bufs=1, space="DRAM") as dram,
        ):
            # ---- resident tiles ----
            gut_sb = singles.tile([P, 2 * FT, HT, P], F32R)   # 64KB/part
            dpt_sb = singles.tile([P, FT, H], F32R)           # 32KB/part
            wrt_sb = singles.tile([P, HT, E], F32)
            xme_sb = singles.tile([P, HT, ME], F32)
            sel_sb = singles.tile([P, E], F32)
            ident = singles.tile([P, P], F32)

            # one y tile per chunk: a shared tensor would put a false
            # (tensor-level) dependency between chunk c's ReduceScatter read
            # and chunk c+1's output writes
            y_drams = [dram.tile([CN, H], F32, name=f"y{c}", tag=f"y{c}")
                       for c in range(NCHUNK)]
            lg_in = dram.tile([E, ME], F32)
            lg_out = dram.tile([8 * E, ME], F32)
            rs_outs = [dram.tile([CN // 8, H], F32, name=f"rs{c}", tag=f"rs{c}")
                       for c in range(NCHUNK)]

            xtr_tiles = [xtr_pool.tile([P, HT, CN], F32R, name=f"xtr{c}",
                                       tag="xtr") for c in range(NCHUNK)]

            def load_chunk(c):
                nsl = slice(c * CN, (c + 1) * CN)
                for h in range(HT):
                    nc.sync.dma_start(out=xtr_tiles[c][:, h, :],
                                      in_=xt_r[h][:, nsl].bitcast(F32R))

            # ---- loads (order matters for early PE start) ----
            for h in range(HT):
                nc.sync.dma_start(out=wrt_sb[:, h, :], in_=wrt_r[h])
                nc.sync.dma_start(out=xme_sb[:, h, :], in_=xme_r[h])
            nc.sync.dma_start(out=sel_sb, in_=sel[:, :])
            # first gate/up weight tiles before chunk-0 X so the first
            # gate matmul group isn't starved after the router matmuls
            nc.sync.dma_start(out=gut_sb[:, 0, :, :], in_=gut[0])
            nc.sync.dma_start(out=gut_sb[:, FT, :, :], in_=gut[FT])
            load_chunk(0)
            make_identity(nc, ident)
            # remaining gate/up tiles interleaved in consumption order:
            # the f-th gate/up matmul group needs tiles (f, FT+f).
            for f in range(1, FT):
                for ft in (f, FT + f):
                    nc.sync.dma_start(out=gut_sb[:, ft, :, :], in_=gut[ft])
            for f in range(FT):
                nc.sync.dma_start(out=dpt_sb[:, f, :], in_=dpt_r[f])

            # ---- sharded router: my 256 tokens, fp32 ----
            # (borrows a gate-psum slot; router precedes all gate matmuls)
            ps_r = r_pool.tile([E, ME], F32)
            for h in range(HT):
                nc.tensor.matmul(ps_r, wrt_sb[:, h, :], xme_sb[:, h, :],
                                 start=(h == 0), stop=(h == HT - 1))
            lme = rp.tile([E, ME], F32)
            nc.vector.tensor_copy(lme, ps_r)
            nc.sync.dma_start(out=lg_in[:, :], in_=lme)
            nc.gpsimd.collective_compute(
                "AllGather", mybir.AluOpType.bypass,
                replica_groups=[list(range(8))],
                ins=[lg_in[:, :].opt()], outs=[lg_out[:, :].opt()])
            # transpose to token-major ltok [P, NT, E]; logits for token
            # slice s loaded straight from the AllGather result
            lg_r = lg_out.rearrange("(j e) m -> j e m", e=E)
            ltok = rp.tile([P, NT, E], F32)
            for s in range(NT):
                lf_s = lf_pool.tile([E, P], F32, name=f"lf{s}", tag="lf")
                m0 = (s % 2) * P
                nc.sync.dma_start(out=lf_s, in_=lg_r[s // 2, :, m0:m0 + P])
                ps_t = t_pool.tile([P, E], F32)
                nc.tensor.transpose(ps_t, lf_s, ident[:E, :E])
                nc.vector.tensor_copy(ltok[:, s, :], ps_t)

            # ---- top-2 + renormalized combine weight for my expert ----
            selb = bass.AP(tensor=sel_sb.tensor, offset=sel_sb.offset,
                           ap=[sel_sb.ap[0], [0, NT], sel_sb.ap[1]])
            lsel = rp.tile([P, NT, E], F32)
            nc.vector.tensor_mul(lsel, ltok, selb)
            l0 = rp.tile([P, NT], F32)
            nc.vector.reduce_sum(l0, lsel, axis=mybir.AxisListType.X)
            m1 = rp.tile([P, NT], F32)
            nc.vector.reduce_max(m1, ltok, axis=mybir.AxisListType.X)
            m1b = bass.AP(tensor=m1.tensor, offset=m1.offset,
                          ap=[m1.ap[0], m1.ap[1], [0, E]])
            eq = rp.tile([P, NT, E], F32)
            nc.vector.tensor_tensor(eq, ltok, m1b, mybir.AluOpType.is_equal)
            masked = rp.tile([P, NT, E], F32)
            nc.vector.scalar_tensor_tensor(masked, eq, -1e30, ltok,
                                           mybir.AluOpType.mult,
                                           mybir.AluOpType.add)
            m2 = rp.tile([P, NT], F32)
            nc.vector.reduce_max(m2, masked, axis=mybir.AxisListType.X)
            ge = rp.tile([P, NT], F32)
            nc.vector.tensor_tensor(ge, l0, m2, mybir.AluOpType.is_ge)
            d1 = rp.tile([P, NT], F32)
            nc.vector.tensor_sub(d1, l0, m1)
            e1 = rp.tile([P, NT], F32)
            nc.scalar.activation(e1, d1, mybir.ActivationFunctionType.Exp)
            d2 = rp.tile([P, NT], F32)
            nc.vector.tensor_sub(d2, m2, m1)
            t2 = rp.tile([P, NT], F32)
            nc.scalar.activation(t2, d2, mybir.ActivationFunctionType.Exp)
            den = rp.tile([P, NT], F32)
            nc.vector.tensor_scalar_add(den, t2, 1.0)
            rec = rp.tile([P, NT], F32)
            nc.vector.reciprocal(rec, den)
            w = rp.tile([P, NT], F32)
            nc.vector.tensor_mul(w, e1, ge)
            nc.vector.tensor_mul(w, w, rec)

            # ---- FFN over all tokens, chunked ----
            for c in range(NCHUNK):
                if c + 1 < NCHUNK:
                    load_chunk(c + 1)
                xtr = xtr_tiles[c]

                act_c = act_pool.tile([P, FT, CN], F32R, name=f"act{c}", tag="act")
                for f in range(FT):
                    ps_g = g_pool.tile([P, CN], F32, name="ps_g", tag="ps_g")
                    for h in range(HT):
                        nc.tensor.matmul(ps_g, gut_sb[:, f, h, :], xtr[:, h, :],
                                         start=(h == 0), stop=(h == HT - 1))
                    ps_u = u_pool.tile([P, CN], F32, name="ps_u", tag="ps_u")
                    for h in range(HT):
                        nc.tensor.matmul(ps_u, gut_sb[:, FT + f, h, :], xtr[:, h, :],
                                         start=(h == 0), stop=(h == HT - 1))
                    sg = sg_pool.tile([P, CN], F32R)
                    nc.scalar.activation(sg, ps_g, mybir.ActivationFunctionType.Silu)
                    nc.vector.tensor_mul(act_c[:, f, :], sg, ps_u)

                for s in range(NSUB):
                    for hc in range(2):
                        ps_d = d_pool.tile([P, 512], F32)
                        for f in range(FT):
                            nc.tensor.matmul(ps_d,
                                             act_c[:, f, s * P:(s + 1) * P],
                                             dpt_sb[:, f, hc * 512:(hc + 1) * 512],
                                             start=(f == 0), stop=(f == FT - 1))
                        y_sb = y_pool.tile([P, 512], F32)
                        nc.scalar.mul(y_sb, ps_d, w[:, c * NSUB + s:c * NSUB + s + 1])
                        row0 = s * P
                        nc.gpsimd.dma_start(
                            out=y_drams[c][row0:row0 + P, hc * 512:(hc + 1) * 512],
                            in_=y_sb)

                nc.gpsimd.collective_compute(
                    "ReduceScatter",
                    mybir.AluOpType.add,
                    replica_groups=[list(range(8))],
                    ins=[y_drams[c][:, :].opt()],
                    outs=[rs_outs[c][:, :].opt()],
                )
                nc.sync.dma_start(out=out[c * (CN // 8):(c + 1) * (CN // 8), :],
                                  in_=rs_outs[c][:, :])

    nc.finalize()
    return nc


_CACHE = {}


def _get_nc():
    if "nc" not in _CACHE:
        _CACHE["nc"] = build_nc()
    return _CACHE["nc"]


def _make_in_maps(hidden_states, router_weight, gate_up_proj, down_proj):
    hs = np.asarray(hidden_states, dtype=np.float32)
    rw = np.asarray(router_weight, dtype=np.float32)
    gu = np.asarray(gate_up_proj, dtype=np.float32)
    dp = np.asarray(down_proj, dtype=np.float32)
    x = hs.reshape(-1, hs.shape[-1])
    xt = np.ascontiguousarray(x.T)
    wrt_t = np.ascontiguousarray(rw.T)
    in_maps = []
    for e in range(8):
        gut_t = np.ascontiguousarray(
            gu[e].reshape(2 * FT, P, HT, P).transpose(0, 3, 2, 1))
        sel = np.zeros((P, E), dtype=np.float32)
        sel[:, e] = 1.0
        in_maps.append({
            "xt": xt,
            "xme": np.ascontiguousarray(xt[:, e * ME:(e + 1) * ME]),
            "wrt": wrt_t,
            "sel": sel,
            "gut": gut_t,
            "dpt": np.ascontiguousarray(dp[e].T),
        })
    return in_maps, hs.shape


def _unshard(results, shape):
    """Core j's output rows [64*c : 64*(c+1)) are global rows
    [512*c + 64*j, 512*c + 64*(j+1))."""
    full = np.empty((N, H), dtype=np.float32)
    m = CN // 8  # 64 rows per (chunk, core)
    for j in range(8):
        o = results[j]["out"]
        for c in range(NCHUNK):
            full[c * CN + j * m:c * CN + (j + 1) * m] = o[c * m:(c + 1) * m]
    return full.reshape(shape)


def kernel(hidden_states, router_weight, gate_up_proj, down_proj):
    in_maps, shape = _make_in_maps(hidden_states, router_weight,
                                   gate_up_proj, down_proj)
    res = run_bass_kernel_spmd(_get_nc(), in_maps, list(range(8))).results
    return _unshard(res, shape)
